# revision 7
# baseline (speedup 1.0000x reference)
"""BitNet FFN (b1.58) Trainium2 kernel — 8-way Megatron tensor-parallel.

Strategy (hardcoded for x:[4,2048,2048], w_gate/w_up:[8192,2048],
w_down:[2048,8192], subln_weight:[8192], fp32):

  - Shard the intermediate dim I=8192 over 8 cores (I_loc=1024):
    w_gate/w_up row-shards, w_down column-shard, subln_weight shard.
    x is replicated; every core processes all 8192 tokens.
  - All quantization happens on device, matching the reference exactly:
      * activation quant: per-token scale 127/clip(absmax, 1e-5); round =
        fp32 RNE via the +/-1.5*2^23 magic-constant trick (== jnp.round);
        values never need clipping (|x*scale| <= 127 by construction).
      * weight quant: per-tensor scale 1/clip(mean|w|, 1e-5); global mean
        via a 3-scalar AllReduce of per-shard |w| sums.
  - Matmuls run on integer-valued bf16 activations x fp8 ternary weights
    with fp32 PSUM accumulation — bit-exact integer arithmetic (sums are
    < 2^24); dequant scales are folded into the PSUM-drain passes.
  - subln needs per-token sum(A^2) and absmax over the full I: per-chunk
    AllReduce(add) + AllReduce(max) of [512]-float stats.
  - The down-projection partial sums are combined with a per-chunk
    ReduceScatter over tokens; the host only re-concatenates shards.
"""
import sys

sys.path.insert(0, "/opt/trn_rl_repo")

import numpy as np

F32 = None  # set lazily in _build (mybir import)

H = 2048
I = 8192
N_CORES = 8
T_TOTAL = 8192
CHUNK = 512
EPS = 1e-5
LN_EPS = 1e-6
C_MAGIC = 12582912.0  # 1.5 * 2**23

_CACHE = {}


def build_nc(h=H, i_full=I, n_cores=N_CORES, t_total=T_TOTAL, chunk=CHUNK):
    from concourse import bacc, tile, mybir

    F32 = mybir.dt.float32
    BF16 = mybir.dt.bfloat16
    FP8 = mybir.dt.float8e4
    AF = mybir.ActivationFunctionType
    ALU = mybir.AluOpType
    AX = mybir.AxisListType

    i_loc = i_full // n_cores
    kh = h // 128            # contraction tiles for gate/up
    si = i_loc // 128        # contraction tiles for down
    tt_n = chunk // 128      # token tiles per chunk
    nch = t_total // chunk   # chunks
    wi_gu = min(512, i_loc)  # gate/up psum width
    ni = i_loc // wi_gu
    wi_d = min(512, h)       # down psum width
    nh = h // wi_d
    rs_sh = chunk // n_cores
    inv_elems = 1.0 / (i_full // n_cores * h)  # per-shard elem count (mean uses full: see below)
    # mean|w| = global_sum / (i_full*h); global_sum = AR over shards
    inv_welems = 1.0 / (i_full * h)
    groups = [list(range(n_cores))]

    nc = bacc.Bacc("TRN2", target_bir_lowering=False, debug=False,
                   num_devices=n_cores)

    x_in = nc.dram_tensor("x", [t_total, h], F32, kind="ExternalInput").ap()
    wg_in = nc.dram_tensor("wg", [i_loc, h], F32, kind="ExternalInput").ap()
    wu_in = nc.dram_tensor("wu", [i_loc, h], F32, kind="ExternalInput").ap()
    wd_in = nc.dram_tensor("wd", [h, i_loc], F32, kind="ExternalInput").ap()
    g_in = nc.dram_tensor("g", [1, i_loc], F32, kind="ExternalInput").ap()
    out_ext = nc.dram_tensor("out", [nch * rs_sh, h], F32,
                             kind="ExternalOutput").ap()

    with tile.TileContext(nc) as tc:
        with (
            tc.tile_pool(name="res", bufs=1) as res,       # persistent
            tc.tile_pool(name="xw", bufs=3) as xw,         # fp32 [128,h] work
            tc.tile_pool(name="xqw", bufs=3) as xqw,       # bf16 [128,h]
            tc.tile_pool(name="bmid", bufs=3) as bmid,     # bf16 mid tiles
            tc.tile_pool(name="xt", bufs=2) as xtp,        # xqT
            tc.tile_pool(name="yt", bufs=2) as ytp,        # yqT
            tc.tile_pool(name="zp", bufs=5) as zp,         # z fp32
            tc.tile_pool(name="scr", bufs=3) as scr,       # relu / sq scratch
            tc.tile_pool(name="osb", bufs=3) as osbp,      # out staging
            tc.tile_pool(name="sm", bufs=10) as sm,        # small [128,k]
            tc.tile_pool(name="stat", bufs=4) as statp,
            tc.tile_pool(name="psgu", bufs=6, space="PSUM") as psgu,
            tc.tile_pool(name="psd", bufs=2, space="PSUM") as psd,
            tc.tile_pool(name="dram", bufs=2, space="DRAM") as dramp,
            tc.tile_pool(name="dram1", bufs=1, space="DRAM") as dram1,
        ):
            # ================= weight scales =================
            acc3 = res.tile([128, 3], F32)
            nc.gpsimd.memset(acc3[:], 0.0)
            w_list = [(wg_in, i_loc), (wu_in, i_loc), (wd_in, h)]
            for idx, (w_ap, rows) in enumerate(w_list):
                for t in range(rows // 128):
                    wt = xw.tile([128, w_ap.shape[1]], F32, tag="xw")
                    nc.sync.dma_start(wt[:], w_ap[t * 128:(t + 1) * 128, :])
                    sct = scr.tile([128, wi_gu], mybir.dt.bfloat16, tag="scr")
                    pacc = sm.tile([128, 1], F32, tag="pacc")
                    # Abs with free-dim accumulate; scratch out is garbage
                    nc.scalar.activation(sct[:], wt[:, 0:wi_gu], AF.Abs,
                                         accum_out=pacc[:])
                    # remaining columns (if tile wider than scratch)
                    col = wi_gu
                    while col < w_ap.shape[1]:
                        pacc2 = sm.tile([128, 1], F32, tag="pacc")
                        nc.scalar.activation(sct[:], wt[:, col:col + wi_gu],
                                             AF.Abs, accum_out=pacc2[:])
                        nc.vector.tensor_tensor(pacc[:], pacc[:], pacc2[:],
                                                op=ALU.add)
                        col += wi_gu
                    nc.vector.tensor_tensor(acc3[:, idx:idx + 1],
                                            acc3[:, idx:idx + 1], pacc[:],
                                            op=ALU.add)
            ones = res.tile([128, 1], F32)
            nc.gpsimd.memset(ones[:], 1.0)
            lneps = res.tile([128, 1], F32)
            nc.gpsimd.memset(lneps[:], LN_EPS)
            ps3 = psd.tile([3, 1], F32, tag="pd")
            nc.tensor.matmul(ps3[:], acc3[:], ones[:], start=True, stop=True)
            sums3 = sm.tile([3, 1], F32, tag="s3")
            nc.scalar.copy(sums3[:], ps3[:])
            wsum_d = dram1.tile([3, 1], F32)
            nc.sync.dma_start(wsum_d[:], sums3[:])
            wsum_o = dram1.tile([3, 1], F32)
            nc.gpsimd.collective_compute(
                "AllReduce", mybir.AluOpType.add, replica_groups=groups,
                ins=[wsum_d[:]], outs=[wsum_o[:]])
            w3 = sm.tile([1, 3], F32, tag="w3")
            nc.sync.dma_start(w3[:], wsum_o[:].rearrange("s o -> o s"))
            # mean -> clip -> scale / inv
            nc.vector.tensor_scalar(out=w3[:], in0=w3[:], scalar1=inv_welems,
                                    scalar2=EPS, op0=ALU.mult, op1=ALU.max)
            w3r = sm.tile([1, 3], F32, tag="w3")
            nc.vector.reciprocal(w3r[:], w3[:])  # w scale (1/clip(mean))
            sc_d = dram1.tile([2, 3], F32)
            nc.sync.dma_start(sc_d[0:1, :], w3r[:])
            nc.sync.dma_start(sc_d[1:2, :], w3[:])
            swb = res.tile([128, 2, 3], F32)
            nc.sync.dma_start(
                swb[:], sc_d[:].rearrange("r s -> (r s)")
                .rearrange("(o r s) -> o r s", o=1, r=2).broadcast_to([128, 2, 3]))

            # ================= weight quantize + transpose =================
            wq_drams = []
            for idx, (w_ap, rows) in enumerate(w_list):
                cols = w_ap.shape[1]
                wq_d = dram1.tile([rows, cols], BF16, tag=f"wq{idx}")
                wq_drams.append(wq_d)
                for t in range(rows // 128):
                    wt = xw.tile([128, cols], F32, tag="xw")
                    nc.sync.dma_start(wt[:], w_ap[t * 128:(t + 1) * 128, :])
                    # w*s + C (in-place)
                    nc.scalar.activation(wt[:], wt[:], AF.Copy, bias=C_MAGIC,
                                         scale=swb[:, 0, idx:idx + 1])
                    # clip to [C-1, C+1] (== clip(w*s, -1, 1) + C)
                    nc.vector.tensor_scalar(
                        out=wt[:], in0=wt[:], scalar1=C_MAGIC + 1.0,
                        scalar2=C_MAGIC - 1.0, op0=ALU.min, op1=ALU.max)
                    wqt = xqw.tile([128, cols], BF16, tag="xqw")
                    nc.vector.tensor_scalar_add(wqt[:], wt[:], -C_MAGIC)
                    nc.sync.dma_start(wq_d[t * 128:(t + 1) * 128, :], wqt[:])

            wgqT = res.tile([128, kh, i_loc], FP8)
            wuqT = res.tile([128, kh, i_loc], FP8)
            wdqT = res.tile([128, si, h], FP8)
            for dst, src, nslab, slabw in ((wgqT, wq_drams[0], kh, i_loc),
                                           (wuqT, wq_drams[1], kh, i_loc),
                                           (wdqT, wq_drams[2], si, h)):
                for j in range(nslab):
                    tb = bmid.tile([128, slabw], BF16, tag="bmid")
                    nc.sync.dma_start(tb[:], src[:, j * 128:(j + 1) * 128],
                                      transpose=True)
                    nc.vector.tensor_copy(dst[:, j, :], tb[:])

            g_rep = res.tile([128, i_loc], F32)
            nc.sync.dma_start(g_rep[:], g_in[:].broadcast_to([128, i_loc]))

            # ================= token chunks =================
            for ci in range(nch):
                base = ci * chunk
                invs = sm.tile([128, tt_n], F32, tag="invs")
                xq_d = dramp.tile([chunk, h], BF16, tag="xqd")
                for tt in range(tt_n):
                    xt = xw.tile([128, h], F32, tag="xw")
                    nc.sync.dma_start(
                        xt[:], x_in[base + tt * 128: base + (tt + 1) * 128, :])
                    m = sm.tile([128, 1], F32, tag="m")
                    nc.vector.tensor_reduce(m[:], xt[:], axis=AX.X,
                                            op=ALU.max,
                                            apply_absolute_value=True)
                    nc.vector.tensor_scalar_max(m[:], m[:], EPS)
                    sx = sm.tile([128, 1], F32, tag="sx")
                    nc.vector.reciprocal(sx[:], m[:])
                    nc.vector.tensor_scalar_mul(sx[:], sx[:], 127.0)
                    nc.vector.tensor_scalar_mul(invs[:, tt:tt + 1], m[:],
                                                1.0 / 127.0)
                    nc.scalar.activation(xt[:], xt[:], AF.Copy, bias=C_MAGIC,
                                         scale=sx[:])
                    xq = xqw.tile([128, h], BF16, tag="xqw")
                    nc.vector.tensor_scalar_add(xq[:], xt[:], -C_MAGIC)
                    nc.sync.dma_start(xq_d[tt * 128:(tt + 1) * 128, :], xq[:])
                xqT = xtp.tile([128, kh, chunk], BF16, tag="xqT")
                for j in range(kh):
                    nc.sync.dma_start(xqT[:, j, :],
                                      xq_d[:, j * 128:(j + 1) * 128],
                                      transpose=True)

                st = statp.tile([128, 2 * tt_n], F32, tag="st")
                zs = []
                for tt in range(tt_n):
                    pgs = [psgu.tile([128, wi_gu], F32, tag="gu",
                                     name=f"pg{n}") for n in range(ni)]
                    pus = [psgu.tile([128, wi_gu], F32, tag="gu",
                                     name=f"pu{n}") for n in range(ni)]
                    for k in range(kh):
                        lhs = xqT[:, k, tt * 128:(tt + 1) * 128]
                        for n in range(ni):
                            nc.tensor.matmul(
                                pgs[n][:], lhs,
                                wgqT[:, k, n * wi_gu:(n + 1) * wi_gu],
                                start=(k == 0), stop=(k == kh - 1))
                            nc.tensor.matmul(
                                pus[n][:], lhs,
                                wuqT[:, k, n * wi_gu:(n + 1) * wi_gu],
                                start=(k == 0), stop=(k == kh - 1))
                    z = zp.tile([128, i_loc], F32, tag="z")
                    for n in range(ni):
                        sl = slice(n * wi_gu, (n + 1) * wi_gu)
                        r = scr.tile([128, wi_gu], F32, tag="scr")
                        nc.scalar.activation(r[:], pgs[n][:], AF.Relu)
                        nc.vector.tensor_tensor(z[:, sl], r[:], pus[n][:],
                                                op=ALU.mult)
                        nc.vector.tensor_tensor(z[:, sl], z[:, sl], r[:],
                                                op=ALU.mult)
                    # z currently holds T = U*relu(G)^2 (integer-scaled)
                    sq = scr.tile([128, i_loc], BF16, tag="scr")
                    nc.scalar.activation(sq[:], z[:], AF.Square,
                                         accum_out=st[:, tt:tt + 1])
                    nc.vector.tensor_tensor(z[:], z[:], g_rep[:], op=ALU.mult)
                    nc.vector.tensor_reduce(st[:, tt_n + tt:tt_n + tt + 1],
                                            z[:], axis=AX.X, op=ALU.max,
                                            apply_absolute_value=True)
                    zs.append(z)

                ss_d = dramp.tile([chunk, 1], F32, tag="ssd")
                mz_d = dramp.tile([chunk, 1], F32, tag="mzd")
                nc.sync.dma_start(
                    ss_d[:].rearrange("(p t) o -> p (t o)", t=tt_n),
                    st[:, 0:tt_n])
                nc.sync.dma_start(
                    mz_d[:].rearrange("(p t) o -> p (t o)", t=tt_n),
                    st[:, tt_n:2 * tt_n])
                ss_o = dramp.tile([chunk, 1], F32, tag="sso")
                mz_o = dramp.tile([chunk, 1], F32, tag="mzo")
                nc.gpsimd.collective_compute(
                    "AllReduce", ALU.add, replica_groups=groups,
                    ins=[ss_d[:]], outs=[ss_o[:]])
                nc.gpsimd.collective_compute(
                    "AllReduce", ALU.max, replica_groups=groups,
                    ins=[mz_d[:]], outs=[mz_o[:]])
                ssg = statp.tile([128, tt_n], F32, tag="ssg")
                mzg = statp.tile([128, tt_n], F32, tag="mzg")
                nc.sync.dma_start(
                    ssg[:], ss_o[:].rearrange("(p t) o -> p (t o)", t=tt_n))
                nc.sync.dma_start(
                    mzg[:], mz_o[:].rearrange("(p t) o -> p (t o)", t=tt_n))

                # per-token scalars: a=invs*inv_swg, b=invs*inv_swu, c=a^2*b
                a_t = sm.tile([128, tt_n], F32, tag="a")
                b_t = sm.tile([128, tt_n], F32, tag="b")
                c_t = sm.tile([128, tt_n], F32, tag="c")
                nc.vector.tensor_scalar_mul(a_t[:], invs[:], swb[:, 1, 0:1])
                nc.vector.tensor_scalar_mul(b_t[:], invs[:], swb[:, 1, 1:2])
                nc.vector.tensor_tensor(c_t[:], a_t[:], a_t[:], op=ALU.mult)
                nc.vector.tensor_tensor(c_t[:], c_t[:], b_t[:], op=ALU.mult)
                # var = ssg*c^2/I ; c1 = 1/sqrt(var+LN_EPS)
                v_t = sm.tile([128, tt_n], F32, tag="v")
                nc.vector.tensor_tensor(v_t[:], ssg[:], c_t[:], op=ALU.mult)
                nc.vector.tensor_tensor(v_t[:], v_t[:], c_t[:], op=ALU.mult)
                c1 = sm.tile([128, tt_n], F32, tag="c1")
                nc.scalar.activation(c1[:], v_t[:], AF.Sqrt,
                                     bias=lneps[:], scale=1.0 / i_full)
                nc.vector.reciprocal(c1[:], c1[:])
                # ymax = mzg*c*c1 ; st = 127/clip(ymax,EPS); inv_st
                ym = sm.tile([128, tt_n], F32, tag="ym")
                nc.vector.tensor_tensor(ym[:], mzg[:], c_t[:], op=ALU.mult)
                nc.vector.tensor_tensor(ym[:], ym[:], c1[:], op=ALU.mult)
                nc.vector.tensor_scalar_max(ym[:], ym[:], EPS)
                s_t = sm.tile([128, tt_n], F32, tag="stq")
                nc.vector.reciprocal(s_t[:], ym[:])
                nc.vector.tensor_scalar_mul(s_t[:], s_t[:], 127.0)
                os_t = sm.tile([128, tt_n], F32, tag="os")
                nc.vector.tensor_scalar_mul(os_t[:], ym[:], 1.0 / 127.0)
                nc.vector.tensor_scalar_mul(os_t[:], os_t[:], swb[:, 1, 2:3])
                cs = sm.tile([128, tt_n], F32, tag="cs")
                nc.vector.tensor_tensor(cs[:], c_t[:], c1[:], op=ALU.mult)
                nc.vector.tensor_tensor(cs[:], cs[:], s_t[:], op=ALU.mult)

                yq_d = dramp.tile([chunk, i_loc], BF16, tag="yqd")
                for tt in range(tt_n):
                    z = zs[tt]
                    nc.scalar.activation(z[:], z[:], AF.Copy, bias=C_MAGIC,
                                         scale=cs[:, tt:tt + 1])
                    yq = bmid.tile([128, i_loc], BF16, tag="bmid")
                    nc.vector.tensor_scalar_add(yq[:], z[:], -C_MAGIC)
                    nc.sync.dma_start(yq_d[tt * 128:(tt + 1) * 128, :], yq[:])
                yqT = ytp.tile([128, si, chunk], BF16, tag="yqT")
                for s in range(si):
                    nc.sync.dma_start(yqT[:, s, :],
                                      yq_d[:, s * 128:(s + 1) * 128],
                                      transpose=True)

                rs_in = dramp.tile([chunk, h], F32, tag="rsin")
                for tt in range(tt_n):
                    for n in range(nh):
                        pd = psd.tile([128, wi_d], F32, tag="pd")
                        for s in range(si):
                            nc.tensor.matmul(
                                pd[:], yqT[:, s, tt * 128:(tt + 1) * 128],
                                wdqT[:, s, n * wi_d:(n + 1) * wi_d],
                                start=(s == 0), stop=(s == si - 1))
                        ob = osbp.tile([128, wi_d], F32, tag="osb")
                        nc.scalar.activation(ob[:], pd[:], AF.Copy,
                                             scale=os_t[:, tt:tt + 1])
                        nc.sync.dma_start(
                            rs_in[tt * 128:(tt + 1) * 128,
                                  n * wi_d:(n + 1) * wi_d], ob[:])
                rs_out = dramp.tile([rs_sh, h], F32, tag="rsout")
                nc.gpsimd.collective_compute(
                    "ReduceScatter", ALU.add, replica_groups=groups,
                    ins=[rs_in[:]], outs=[rs_out[:]])
                nc.sync.dma_start(out_ext[ci * rs_sh:(ci + 1) * rs_sh, :],
                                  rs_out[:])

    nc.compile()
    return nc


def _get_nc(key, **kw):
    if key not in _CACHE:
        _CACHE[key] = build_nc(**kw)
    return _CACHE[key]


def kernel(x, w_gate, w_up, w_down, subln_weight):
    from concourse.bass_utils import run_bass_kernel_spmd

    nc = _get_nc("full")
    x2 = np.ascontiguousarray(np.asarray(x, np.float32).reshape(T_TOTAL, H))
    i_loc = I // N_CORES
    in_maps = []
    for c in range(N_CORES):
        sl = slice(c * i_loc, (c + 1) * i_loc)
        in_maps.append({
            "x": x2,
            "wg": np.ascontiguousarray(np.asarray(w_gate, np.float32)[sl, :]),
            "wu": np.ascontiguousarray(np.asarray(w_up, np.float32)[sl, :]),
            "wd": np.ascontiguousarray(np.asarray(w_down, np.float32)[:, sl]),
            "g": np.ascontiguousarray(
                np.asarray(subln_weight, np.float32).reshape(1, I)[:, sl]),
        })
    res = run_bass_kernel_spmd(nc, in_maps, list(range(N_CORES)))
    rs_sh = CHUNK // N_CORES
    nch = T_TOTAL // CHUNK
    full = np.empty((nch, N_CORES, rs_sh, H), np.float32)
    for c in range(N_CORES):
        full[:, c] = res.results[c]["out"].reshape(nch, rs_sh, H)
    return full.reshape(4, 2048, H)
